# revision 19
# baseline (speedup 1.0000x reference)
"""Trainium2 Bass kernel for GQA causal multi-head attention.

Problem: x[B=2,S=2048,E=2048], Wq[H=16,E,D=128], Wk/Wv[G=4,E,D] (+biases).
  q = x@Wq+bq ; k,v = x@Wk/Wv+bk/bv (per group, each group serves 4 heads)
  out = causal_softmax(q k^T / sqrt(D)) @ v, heads concatenated.

Sharding: core c in 0..7 -> (batch b = c//4, group g = c%4).
Each core computes Q for heads 4g..4g+3 of batch b, K/V for group g of
batch b, and full causal attention for those 4 heads. Zero duplication,
no collectives; host does the final transpose/normalize/concat.

Per-core kernel (all layouts chosen so no on-chip transposes of big data):
  xT[E,S] streamed; QT[d,s] = Wq^T xT (PSUM accum over E-tiles), same for
  KT, VT.  V obtained from VT via 16 PE transposes (128x128).
  Attention (transposed-scores formulation): for each q-block of 512 and
  k-sub of 128:  ST[k,q] = (KT chunk)^T-contracted matmul, exp on ACT ->
  PT[k,q] directly (no P transpose!), OT[d,q] += V^T-form matmul(PT),
  denominators via ones-vector M=1 matmul. Causal: skip k>q chunks,
  shrink N on diagonal, single 128x128 triangle mask tile.
Outputs: raw OT[h,d,S] + row sums; host computes (OT/sums)^T.
"""

import math
import os
import sys

sys.path.insert(0, "/opt/trn_rl_repo")

import numpy as np
import ml_dtypes

B, S, E, H, G, D = 2, 2048, 2048, 16, 4, 128
PER = H // G          # 4 q-heads per kv group = heads per core
NCORES = 8
QB = 512              # q block (moving-dim) size
SCALE = 1.0 / math.sqrt(D)

# compute dtype for matmul operands: "bfloat16" | "float32r" | "float32"
CDT_NAME = os.environ.get("BASS_CDT", "bfloat16")

_CACHE = {}


def build_nc(s=S, cdt_name=CDT_NAME):
    """Build the per-core Bass program (same program for all 8 cores)."""
    import concourse.bass as bass
    import concourse.tile as tile
    from concourse import bacc, mybir
    from concourse.masks import make_identity
    from contextlib import ExitStack

    cdt = getattr(mybir.dt, cdt_name)
    f32 = mybir.dt.float32
    ET = E // 128          # e-tiles (contraction)
    NSC = s // QB          # s-chunks == q-supers
    KS = s // 128          # k-subs over full sequence
    EXP = mybir.ActivationFunctionType.Exp

    nc = bacc.Bacc("TRN2", target_bir_lowering=False, debug=False,
                   num_devices=NCORES)

    xt_d = nc.dram_tensor("xt", [E, s], cdt, kind="ExternalInput")
    wq_d = nc.dram_tensor("wq", [E, PER * D], cdt, kind="ExternalInput")
    wk_d = nc.dram_tensor("wk", [E, D], cdt, kind="ExternalInput")
    wv_d = nc.dram_tensor("wv", [E, D], cdt, kind="ExternalInput")
    bq_d = nc.dram_tensor("bq", [D, PER], f32, kind="ExternalInput")
    bk_d = nc.dram_tensor("bk", [D, 1], f32, kind="ExternalInput")
    bv_d = nc.dram_tensor("bv", [D, 1], f32, kind="ExternalInput")
    mask_d = nc.dram_tensor("mask", [128, 128], f32, kind="ExternalInput")
    ot_d = nc.dram_tensor("ot", [PER, D, s], f32, kind="ExternalOutput")
    sums_d = nc.dram_tensor("sums", [1, PER * s], f32, kind="ExternalOutput")

    with tile.TileContext(nc) as tc, ExitStack() as ctx:
        consts = ctx.enter_context(tc.tile_pool(name="consts", bufs=1))
        qkv = ctx.enter_context(tc.tile_pool(name="qkv", bufs=1))
        xpool = ctx.enter_context(tc.tile_pool(name="xtiles", bufs=3))
        ptpool = ctx.enter_context(tc.tile_pool(name="pt", bufs=8))
        opool = ctx.enter_context(tc.tile_pool(name="obuf", bufs=3))

        # --- constants / weights resident in SBUF
        # Weights are split per e-tile and issued e0-first so the first
        # projection matmuls start as soon as ~200KB (not 3MB) has landed.
        # xt goes on the scalar-engine HWDGE ring, weights on sync's, so
        # the two streams don't queue behind each other.
        wq_sb = consts.tile([128, ET, PER * D], cdt)
        wk_sb = consts.tile([128, ET, D], cdt)
        wv_sb = consts.tile([128, ET, D], cdt)
        wq_r = wq_d[:, :].rearrange("(eo p) m -> p eo m", p=128)
        wk_r = wk_d[:, :].rearrange("(eo p) m -> p eo m", p=128)
        wv_r = wv_d[:, :].rearrange("(eo p) m -> p eo m", p=128)
        for e in range(ET):
            nc.sync.dma_start(wq_sb[:, e, :], wq_r[:, e, :])
            nc.sync.dma_start(wk_sb[:, e, :], wk_r[:, e, :])
            nc.sync.dma_start(wv_sb[:, e, :], wv_r[:, e, :])
        bq_sb = consts.tile([128, PER], f32)
        nc.sync.dma_start(bq_sb[:], bq_d[:, :])
        bk_sb = consts.tile([128, 1], f32)
        nc.sync.dma_start(bk_sb[:], bk_d[:, :])
        bv_sb = consts.tile([128, 1], f32)
        nc.sync.dma_start(bv_sb[:], bv_d[:, :])
        mask_sb = consts.tile([128, 128], f32)
        nc.sync.dma_start(mask_sb[:], mask_d[:, :])
        f32r = mybir.dt.float32r
        ones32_sb = consts.tile([128, 1], f32r)
        nc.vector.memset(ones32_sb[:].bitcast(f32), 1.0)
        ident = consts.tile([128, 128], cdt)
        make_identity(nc, ident[:])

        qt_sb = qkv.tile([128, PER, s], cdt)      # QT per head [d, s]
        kt_sb = qkv.tile([128, s], cdt)           # KT [d, s]
        vt_sb = qkv.tile([128, s], cdt)           # VT [d, s]
        v_sb = qkv.tile([128, s], cdt)            # V k-sub j at [:, j*128:...]
        sums_sb = qkv.tile([1, PER * s], f32)
        # persistent per-(h,qi) exp-sum accumulators; contracted at the end
        acc_sb = qkv.tile([128, PER * NSC, QB], mybir.dt.float32r)

        # --- phase 0: PE warm-up. Dummy matmuls on the (locally generated)
        # identity keep the PE busy while the initial DMAs land, and drive
        # the HAM clock gate to full speed before the real work starts.
        with tc.tile_pool(name="wpsum", bufs=1, space="PSUM") as wpsum:
            warm = wpsum.tile([128, QB], f32, name="warm")
            nwarm = 56
            for w in range(nwarm):
                nc.tensor.matmul(warm[:, :128], ident[:], ident[:],
                                 start=(w == 0), stop=(w == nwarm - 1))

        # --- phase 1: QKV projections (PSUM accumulate over 16 e-tiles)
        # K/V PSUM drains go to the (otherwise idle) scalar engine, Q to
        # DVE, so the per-chunk drain doesn't serialize on one engine.
        with tc.tile_pool(name="ppsum", bufs=8, space="PSUM") as ppsum:
            IDF = mybir.ActivationFunctionType.Identity
            for sc in range(NSC):
                pq = [ppsum.tile([128, QB], f32, tag="pp", name=f"pq{h}")
                      for h in range(PER)]
                pk = ppsum.tile([128, QB], f32, tag="pp")
                pv = ppsum.tile([128, QB], f32, tag="pp")
                xt_t = xpool.tile([128, ET, QB], cdt, tag="xt")
                nc.scalar.dma_start(
                    xt_t[:],
                    xt_d[:, sc * QB:(sc + 1) * QB].rearrange(
                        "(eo p) q -> p eo q", p=128))
                for e in range(ET):
                    st_, sp_ = (e == 0), (e == ET - 1)
                    for h in range(PER):
                        nc.tensor.matmul(pq[h][:], wq_sb[:, e, h * D:(h + 1) * D],
                                         xt_t[:, e, :], start=st_, stop=sp_)
                    nc.tensor.matmul(pk[:], wk_sb[:, e, :], xt_t[:, e, :],
                                     start=st_, stop=sp_)
                    nc.tensor.matmul(pv[:], wv_sb[:, e, :], xt_t[:, e, :],
                                     start=st_, stop=sp_)
                cols = slice(sc * QB, (sc + 1) * QB)
                for h in range(PER):
                    nc.vector.tensor_scalar_add(qt_sb[:, h, cols], pq[h][:],
                                                bq_sb[:, h:h + 1])
                nc.scalar.activation(kt_sb[:, cols], pk[:], IDF,
                                     bias=bk_sb[:, 0:1])
                nc.scalar.activation(vt_sb[:, cols], pv[:], IDF,
                                     bias=bv_sb[:, 0:1])

        # --- phase 1.5: V = VT^T via PE transposes
        with tc.tile_pool(name="tpsum", bufs=4, space="PSUM") as tpsum:
            for j in range(KS):
                vps = tpsum.tile([128, 128], cdt, tag="vt")
                nc.tensor.transpose(vps[:], vt_sb[:, j * 128:(j + 1) * 128],
                                    ident[:])
                nc.vector.tensor_copy(v_sb[:, j * 128:(j + 1) * 128], vps[:])

        # --- phase 2: causal attention, transposed-scores formulation.
        # Off-diagonal k-subs are processed in pairs sharing a 2-bank ST
        # PSUM tile so one ACT exp covers 1024 columns. Softmax
        # denominators: exp tiles are accumulated on DVE into acc_sb and
        # contracted with a ones vector in one f32r matmul per q-block
        # (keeps the M=1 sum matmuls off the PE critical path).
        with tc.tile_pool(name="spsum", bufs=3, space="PSUM") as spsum, \
             tc.tile_pool(name="opsum", bufs=2, space="PSUM") as opsum:
            for h in range(PER):
                for qi in range(NSC):
                    nj = (qi + 1) * (QB // 128)   # k-subs up to diagonal
                    ndiag = QB // 128
                    ot_ps = opsum.tile([128, QB], f32, tag="ot")
                    acc = acc_sb[:, h * NSC + qi, :]
                    # exp-sum chains alternate DVE / (otherwise idle) GpSimd
                    eng = nc.gpsimd if qi % 2 == 0 else nc.vector
                    qsl = qt_sb[:, h, qi * QB:(qi + 1) * QB]

                    def pv(j, pt_ap, qoff, nq):
                        nc.tensor.matmul(ot_ps[:, qoff:QB],
                                         v_sb[:, j * 128:(j + 1) * 128],
                                         pt_ap,
                                         start=(j == 0), stop=(j == nj - 1))

                    def accum(j, pt_ap, qoff, nq):
                        if j == 0:
                            eng.tensor_copy(acc, pt_ap)
                        else:
                            eng.tensor_add(acc[:, qoff:QB],
                                           acc[:, qoff:QB], pt_ap)

                    for jp in range(qi * (QB // 256)):  # off-diag pairs
                        j0 = jp * 2
                        st2 = spsum.tile([128, 2, QB], f32, tag="st")
                        for m in range(2):
                            nc.tensor.matmul(st2[:, m, :],
                                             kt_sb[:, (j0 + m) * 128:
                                                   (j0 + m + 1) * 128],
                                             qsl, start=True, stop=True)
                        pt = ptpool.tile([128, 2, QB], cdt, tag="pt")
                        nc.scalar.activation(pt[:, :, :], st2[:, :, :], EXP,
                                             scale=SCALE)
                        for m in range(2):
                            pv(j0 + m, pt[:, m, :], 0, QB)
                            accum(j0 + m, pt[:, m, :], 0, QB)
                    for jd in range(ndiag):             # diagonal singles
                        j = qi * ndiag + jd
                        kk = j * 128
                        qoff = jd * 128
                        nq = QB - qoff
                        st2 = spsum.tile([128, 2, QB], f32, tag="st")
                        nc.tensor.matmul(st2[:, 0, :nq],
                                         kt_sb[:, kk:kk + 128],
                                         qt_sb[:, h, kk:(qi + 1) * QB],
                                         start=True, stop=True)
                        nc.vector.tensor_add(st2[:, 0, 0:128],
                                             st2[:, 0, 0:128], mask_sb[:])
                        pt = ptpool.tile([128, 2, QB], cdt, tag="pt")
                        nc.scalar.activation(pt[:, 0, :nq], st2[:, 0, :nq],
                                             EXP, scale=SCALE)
                        pv(j, pt[:, 0, :nq], qoff, nq)
                        accum(j, pt[:, 0, :nq], qoff, nq)

                    ot_sbuf = opool.tile([128, QB], f32, tag="osb")
                    nc.vector.tensor_copy(ot_sbuf[:], ot_ps[:])
                    nc.sync.dma_start(ot_d[h, :, qi * QB:(qi + 1) * QB],
                                      ot_sbuf[:])

        # --- final: contract exp-sum accumulators with ones (f32r matmuls)
        with tc.tile_pool(name="dpsum", bufs=4, space="PSUM") as dpsum:
            for u in range(PER * NSC):
                sm_ps = dpsum.tile([1, QB], f32, tag="sm")
                nc.tensor.matmul(sm_ps[:, :], ones32_sb[:],
                                 acc_sb[:, u, :], start=True, stop=True)
                nc.vector.tensor_copy(
                    sums_sb[0:1, u * QB:(u + 1) * QB], sm_ps[0:1, :])
        nc.sync.dma_start(sums_d[:, :], sums_sb[:])

    nc.compile()
    return nc


def _np_cdt(cdt_name):
    return {"bfloat16": ml_dtypes.bfloat16,
            "float32r": np.float32,
            "float32": np.float32}[cdt_name]


def make_in_maps(x, Wq, bq, Wk, bk, Wv, bv, s=S, cdt_name=CDT_NAME):
    """Host-side shard + relayout: per-core input dicts."""
    ndt = _np_cdt(cdt_name)
    x = np.asarray(x, dtype=np.float32)
    Wq = np.asarray(Wq, dtype=np.float32)
    bq = np.asarray(bq, dtype=np.float32)
    Wk = np.asarray(Wk, dtype=np.float32)
    bk = np.asarray(bk, dtype=np.float32)
    Wv = np.asarray(Wv, dtype=np.float32)
    bv = np.asarray(bv, dtype=np.float32)
    mask = np.where(np.arange(128)[:, None] > np.arange(128)[None, :],
                    -1e30, 0.0).astype(np.float32)
    in_maps = []
    xt_b = [np.ascontiguousarray(x[b, :s].T).astype(ndt) for b in range(B)]
    for c in range(NCORES):
        b, g = c // G, c % G
        hs = slice(g * PER, (g + 1) * PER)
        wq_c = np.ascontiguousarray(
            Wq[hs].transpose(1, 0, 2).reshape(E, PER * D)).astype(ndt)
        in_maps.append({
            "xt": xt_b[b],
            "wq": wq_c,
            "wk": np.ascontiguousarray(Wk[g]).astype(ndt),
            "wv": np.ascontiguousarray(Wv[g]).astype(ndt),
            "bq": np.ascontiguousarray(bq[hs].T),
            "bk": np.ascontiguousarray(bk[g][:, None]),
            "bv": np.ascontiguousarray(bv[g][:, None]),
            "mask": mask,
        })
    return in_maps


def assemble(results, s=S):
    """Host-side gather: normalize, transpose, concat heads."""
    out = np.empty((B, s, H * D), dtype=np.float32)
    for c in range(NCORES):
        b, g = c // G, c % G
        ot = results[c]["ot"]                       # [PER, D, s]
        sums = results[c]["sums"].reshape(PER, s)   # [PER, s]
        o = (ot / sums[:, None, :]).transpose(2, 0, 1)   # [s, PER, D]
        out[b, :, g * PER * D:(g + 1) * PER * D] = o.reshape(s, PER * D)
    return out


def _install_trace_shims():
    """Make trace=True work in this container: provide the missing
    antenv.axon_hooks module (ctypes NTFF hook) and disable the
    artifact upload (no bucket access here)."""
    import types
    try:
        import antenv.axon_hooks  # noqa: F401
        have = True
    except ImportError:
        have = False
    if not have:
        from trn_agent_boot.trn_boot import _ntff_profile_via_ctypes
        mod = types.ModuleType("antenv.axon_hooks")
        _hook = _ntff_profile_via_ctypes("/opt/axon/libaxon_pjrt.so")
        mod.get_axon_ntff_profile_hook = lambda: _hook
        mod.set_axon_ntff_profile_hook = lambda h: None
        import antenv
        sys.modules["antenv.axon_hooks"] = mod
        antenv.axon_hooks = mod
    from concourse import bass_utils as bu
    bu.upload_artifacts = lambda tmpdir: f"file://{tmpdir}"


def run(inputs, trace=False, s=S, cdt_name=CDT_NAME):
    from concourse.bass_utils import run_bass_kernel_spmd
    if trace:
        _install_trace_shims()
    key = (s, cdt_name)
    if key not in _CACHE:
        _CACHE[key] = build_nc(s, cdt_name)
    nc = _CACHE[key]
    in_maps = make_in_maps(**inputs, s=s, cdt_name=cdt_name)
    res = run_bass_kernel_spmd(nc, in_maps, list(range(NCORES)), trace=trace)
    return assemble(res.results, s), res


def kernel(**inputs):
    out, _ = run(inputs)
    return out


# revision 22
# speedup vs baseline: 1.0384x; 1.0384x over previous
"""Trainium2 Bass kernel for GQA causal multi-head attention.

Problem: x[B=2,S=2048,E=2048], Wq[H=16,E,D=128], Wk/Wv[G=4,E,D] (+biases).
  q = x@Wq+bq ; k,v = x@Wk/Wv+bk/bv (per group, each group serves 4 heads)
  out = causal_softmax(q k^T / sqrt(D)) @ v, heads concatenated.

Sharding: core c in 0..7 -> (batch b = c//4, group g = c%4).
Each core computes Q for heads 4g..4g+3 of batch b, K/V for group g of
batch b, and full causal attention for those 4 heads. Zero duplication,
no collectives; host does the final transpose/normalize/concat.

Per-core kernel (all layouts chosen so no on-chip transposes of big data):
  xT[E,S] streamed; QT[d,s] = Wq^T xT (PSUM accum over E-tiles), same for
  KT, VT.  V obtained from VT via 16 PE transposes (128x128).
  Attention (transposed-scores formulation): for each q-block of 512 and
  k-sub of 128:  ST[k,q] = (KT chunk)^T-contracted matmul, exp on ACT ->
  PT[k,q] directly (no P transpose!), OT[d,q] += V^T-form matmul(PT),
  denominators via ones-vector M=1 matmul. Causal: skip k>q chunks,
  shrink N on diagonal, single 128x128 triangle mask tile.
Outputs: raw OT[h,d,S] + row sums; host computes (OT/sums)^T.
"""

import math
import os
import sys

sys.path.insert(0, "/opt/trn_rl_repo")

import numpy as np
import ml_dtypes

B, S, E, H, G, D = 2, 2048, 2048, 16, 4, 128
PER = H // G          # 4 q-heads per kv group = heads per core
NCORES = 8
QB = 512              # q block (moving-dim) size
SCALE = 1.0 / math.sqrt(D)

# compute dtype for matmul operands: "bfloat16" | "float32r" | "float32"
CDT_NAME = os.environ.get("BASS_CDT", "bfloat16")

_CACHE = {}


def build_nc(s=S, cdt_name=CDT_NAME):
    """Build the per-core Bass program (same program for all 8 cores).

    v5: projection and attention are interleaved per 512-wide s-chunk so
    the scalar engine's exp work overlaps the PE's projection matmuls.
    Projections run in 3 two-bank PSUM passes ({Q0,Q1},{Q2,Q3},{K,V});
    V is transposed via DMA x-bar (bf16) or PE (f32r); softmax
    denominators accumulate on DVE/GpSimd into per-unit f32r tiles and
    are contracted with a ones vector at the very end.
    """
    import concourse.bass as bass
    import concourse.tile as tile
    from concourse import bacc, mybir
    from concourse.masks import make_identity
    from contextlib import ExitStack

    cdt = getattr(mybir.dt, cdt_name)
    f32 = mybir.dt.float32
    f32r = mybir.dt.float32r
    ET = E // 128          # e-tiles (contraction)
    NSC = s // QB          # s-chunks == q-supers
    EXP = mybir.ActivationFunctionType.Exp

    nc = bacc.Bacc("TRN2", target_bir_lowering=False, debug=False,
                   num_devices=NCORES)

    xt_d = nc.dram_tensor("xt", [E, s], cdt, kind="ExternalInput")
    wq_d = nc.dram_tensor("wq", [E, PER * D], cdt, kind="ExternalInput")
    wk_d = nc.dram_tensor("wk", [E, D], cdt, kind="ExternalInput")
    wv_d = nc.dram_tensor("wv", [E, D], cdt, kind="ExternalInput")
    bq_d = nc.dram_tensor("bq", [D, PER], f32, kind="ExternalInput")
    bk_d = nc.dram_tensor("bk", [D, 1], f32, kind="ExternalInput")
    bv_d = nc.dram_tensor("bv", [D, 1], f32, kind="ExternalInput")
    mask_d = nc.dram_tensor("mask", [128, 128], f32, kind="ExternalInput")
    ot_d = nc.dram_tensor("ot", [PER, D, s], f32, kind="ExternalOutput")
    sums_d = nc.dram_tensor("sums", [1, PER * s], f32, kind="ExternalOutput")

    with tile.TileContext(nc) as tc, ExitStack() as ctx:
        consts = ctx.enter_context(tc.tile_pool(name="consts", bufs=1))
        qkv = ctx.enter_context(tc.tile_pool(name="qkv", bufs=1))
        xpool = ctx.enter_context(tc.tile_pool(name="xtiles", bufs=8))
        ptpool = ctx.enter_context(tc.tile_pool(name="pt", bufs=8))
        opool = ctx.enter_context(tc.tile_pool(name="obuf", bufs=3))
        accp = ctx.enter_context(tc.tile_pool(name="accp", bufs=1))

        # --- constants / weights (wq first + split per e-tile: the first
        # projection pass needs only wq; mask early for the first diagonal)
        wq_sb = consts.tile([128, ET, PER * D], cdt)
        wk_sb = consts.tile([128, ET, D], cdt)
        wv_sb = consts.tile([128, ET, D], cdt)
        wq_r = wq_d[:, :].rearrange("(eo p) m -> p eo m", p=128)
        wk_r = wk_d[:, :].rearrange("(eo p) m -> p eo m", p=128)
        wv_r = wv_d[:, :].rearrange("(eo p) m -> p eo m", p=128)
        for e in range(ET):
            nc.sync.dma_start(wq_sb[:, e, :], wq_r[:, e, :])
        mask_sb = consts.tile([128, 128], f32)
        nc.sync.dma_start(mask_sb[:], mask_d[:, :])
        for e in range(ET):
            nc.sync.dma_start(wk_sb[:, e, :], wk_r[:, e, :])
            nc.sync.dma_start(wv_sb[:, e, :], wv_r[:, e, :])
        bq_sb = consts.tile([128, PER], f32)
        nc.sync.dma_start(bq_sb[:], bq_d[:, :])
        bk_sb = consts.tile([128, 1], f32)
        nc.sync.dma_start(bk_sb[:], bk_d[:, :])
        bv_sb = consts.tile([128, 1], f32)
        nc.sync.dma_start(bv_sb[:], bv_d[:, :])
        ones32_sb = consts.tile([128, 1], f32r)
        nc.vector.memset(ones32_sb[:].bitcast(f32), 1.0)
        ident = consts.tile([128, 128], cdt)
        make_identity(nc, ident[:])

        qt_sb = qkv.tile([128, PER, s], cdt)      # QT per head [d, s]
        kt_sb = qkv.tile([128, s], cdt)           # KT [d, s]
        vt_sb = qkv.tile([128, s], cdt)           # VT [d, s]
        v_sb = qkv.tile([128, s], cdt)            # V k-sub j at [:, j*128:...]
        sums_sb = qkv.tile([1, PER * s], f32)
        accs = [accp.tile([128, QB], f32r, name=f"acc{u}")
                for u in range(PER * NSC)]

        # --- PE warm-up: dummy matmuls on the locally generated identity
        # keep PE busy while the first DMAs land and un-throttle the HAM.
        with tc.tile_pool(name="wpsum", bufs=1, space="PSUM") as wpsum:
            warm = wpsum.tile([128, QB], f32, name="warm")
            nwarm = 72
            for w in range(nwarm):
                nc.tensor.matmul(warm[:, :128], ident[:], ident[:],
                                 start=(w == 0), stop=(w == nwarm - 1))

        mainpools = ctx.enter_context(ExitStack())
        ppsum = mainpools.enter_context(tc.tile_pool(name="ppsum", bufs=2,
                                                     space="PSUM"))
        spsum = mainpools.enter_context(tc.tile_pool(name="spsum", bufs=2,
                                                     space="PSUM"))
        opsum = mainpools.enter_context(tc.tile_pool(name="opsum", bufs=2,
                                                     space="PSUM"))

        def attn_unit(h, qi):
            nj = (qi + 1) * (QB // 128)
            ndiag = QB // 128
            ot_ps = opsum.tile([128, QB], f32, tag="ot", name="ot_ps")
            acc = accs[h * NSC + qi][:]
            eng = nc.gpsimd if qi % 2 == 0 else nc.vector
            qsl = qt_sb[:, h, qi * QB:(qi + 1) * QB]

            def pv(j, pt_ap, qoff):
                nc.tensor.matmul(ot_ps[:, qoff:QB],
                                 v_sb[:, j * 128:(j + 1) * 128], pt_ap,
                                 start=(j == 0), stop=(j == nj - 1))

            def accum(j, pt_ap, qoff):
                if j == 0:
                    eng.tensor_copy(acc, pt_ap)
                else:
                    eng.tensor_add(acc[:, qoff:QB], acc[:, qoff:QB], pt_ap)

            for jp in range(qi * (QB // 256)):  # off-diagonal pairs
                j0 = jp * 2
                st2 = spsum.tile([128, 2, QB], f32, tag="st", name="st2")
                for m in range(2):
                    nc.tensor.matmul(
                        st2[:, m, :],
                        kt_sb[:, (j0 + m) * 128:(j0 + m + 1) * 128],
                        qsl, start=True, stop=True)
                pt = ptpool.tile([128, 2, QB], cdt, tag="pt", name="pt2")
                nc.scalar.activation(pt[:, :, :], st2[:, :, :], EXP,
                                     scale=SCALE)
                for m in range(2):
                    pv(j0 + m, pt[:, m, :], 0)
                    accum(j0 + m, pt[:, m, :], 0)
            for jd in range(ndiag):             # diagonal singles
                j = qi * ndiag + jd
                kk = j * 128
                qoff = jd * 128
                nq = QB - qoff
                st2 = spsum.tile([128, 2, QB], f32, tag="st", name="st1")
                nc.tensor.matmul(st2[:, 0, :nq], kt_sb[:, kk:kk + 128],
                                 qt_sb[:, h, kk:(qi + 1) * QB],
                                 start=True, stop=True)
                nc.vector.tensor_add(st2[:, 0, 0:128], st2[:, 0, 0:128],
                                     mask_sb[:])
                pt = ptpool.tile([128, 2, QB], cdt, tag="pt", name="pt1")
                nc.scalar.activation(pt[:, 0, :nq], st2[:, 0, :nq], EXP,
                                     scale=SCALE)
                pv(j, pt[:, 0, :nq], qoff)
                accum(j, pt[:, 0, :nq], qoff)
            ot_sbuf = opool.tile([128, QB], f32, tag="osb", name="ot_sbuf")
            nc.vector.tensor_copy(ot_sbuf[:], ot_ps[:])
            nc.sync.dma_start(ot_d[h, :, qi * QB:(qi + 1) * QB], ot_sbuf[:])

        # --- main loop: per s-chunk, projection passes then attention
        PASSES = [("q", 0, 1), ("q", 2, 3), ("kv", None, None)]
        for sc in range(NSC):
            cols = slice(sc * QB, (sc + 1) * QB)
            xt_g = []
            for g4 in range(4):   # 4 e-group chunks of 512KB each
                xg = xpool.tile([128, 4, QB], cdt, tag="xt", name=f"xg{g4}")
                nc.scalar.dma_start(
                    xg[:],
                    xt_d[g4 * 4 * 128:(g4 + 1) * 4 * 128, cols].rearrange(
                        "(eo p) q -> p eo q", p=128))
                xt_g.append(xg)

            def xe(e):
                return xt_g[e // 4][:, e % 4, :]

            for kind, ha, hb in PASSES:
                pa = ppsum.tile([128, QB], f32, tag="pp", name="pa")
                pb = ppsum.tile([128, QB], f32, tag="pp", name="pb")
                for e in range(ET):
                    st_, sp_ = (e == 0), (e == ET - 1)
                    if kind == "q":
                        nc.tensor.matmul(pa[:], wq_sb[:, e, ha * D:(ha + 1) * D],
                                         xe(e), start=st_, stop=sp_)
                        nc.tensor.matmul(pb[:], wq_sb[:, e, hb * D:(hb + 1) * D],
                                         xe(e), start=st_, stop=sp_)
                    else:
                        nc.tensor.matmul(pa[:], wk_sb[:, e, :], xe(e),
                                         start=st_, stop=sp_)
                        nc.tensor.matmul(pb[:], wv_sb[:, e, :], xe(e),
                                         start=st_, stop=sp_)
                if kind == "q":
                    nc.vector.tensor_scalar_add(qt_sb[:, ha, cols], pa[:],
                                                bq_sb[:, ha:ha + 1])
                    nc.vector.tensor_scalar_add(qt_sb[:, hb, cols], pb[:],
                                                bq_sb[:, hb:hb + 1])
                else:
                    nc.vector.tensor_scalar_add(kt_sb[:, cols], pa[:],
                                                bk_sb[:, 0:1])
                    nc.vector.tensor_scalar_add(vt_sb[:, cols], pb[:],
                                                bv_sb[:, 0:1])

            # V tiles for this chunk: 4x 128x128 transposes
            if cdt_name == "bfloat16":
                for jd in range(QB // 128):
                    j = sc * (QB // 128) + jd
                    nc.sync.dma_start(v_sb[:, j * 128:(j + 1) * 128],
                                      vt_sb[:, j * 128:(j + 1) * 128],
                                      transpose=True)
            else:
                for jd in range(QB // 128):
                    j = sc * (QB // 128) + jd
                    vps = ppsum.tile([128, 128], cdt, tag="pp", name="vps")
                    nc.tensor.transpose(vps[:],
                                        vt_sb[:, j * 128:(j + 1) * 128],
                                        ident[:])
                    nc.vector.tensor_copy(v_sb[:, j * 128:(j + 1) * 128],
                                          vps[:])

            for h in range(PER):
                attn_unit(h, qi=sc)

        # --- final: contract exp-sum accumulators with ones (f32r matmuls)
        mainpools.close()
        with tc.tile_pool(name="dpsum", bufs=4, space="PSUM") as dpsum:
            for u in range(PER * NSC):
                sm_ps = dpsum.tile([1, QB], f32, tag="sm", name="sm_ps")
                nc.tensor.matmul(sm_ps[:, :], ones32_sb[:], accs[u][:],
                                 start=True, stop=True)
                nc.vector.tensor_copy(sums_sb[0:1, u * QB:(u + 1) * QB],
                                      sm_ps[0:1, :])
        nc.sync.dma_start(sums_d[:, :], sums_sb[:])

    nc.compile()
    return nc


def _np_cdt(cdt_name):
    return {"bfloat16": ml_dtypes.bfloat16,
            "float32r": np.float32,
            "float32": np.float32}[cdt_name]


def make_in_maps(x, Wq, bq, Wk, bk, Wv, bv, s=S, cdt_name=CDT_NAME):
    """Host-side shard + relayout: per-core input dicts."""
    ndt = _np_cdt(cdt_name)
    x = np.asarray(x, dtype=np.float32)
    Wq = np.asarray(Wq, dtype=np.float32)
    bq = np.asarray(bq, dtype=np.float32)
    Wk = np.asarray(Wk, dtype=np.float32)
    bk = np.asarray(bk, dtype=np.float32)
    Wv = np.asarray(Wv, dtype=np.float32)
    bv = np.asarray(bv, dtype=np.float32)
    mask = np.where(np.arange(128)[:, None] > np.arange(128)[None, :],
                    -1e30, 0.0).astype(np.float32)
    in_maps = []
    xt_b = [np.ascontiguousarray(x[b, :s].T).astype(ndt) for b in range(B)]
    for c in range(NCORES):
        b, g = c // G, c % G
        hs = slice(g * PER, (g + 1) * PER)
        wq_c = np.ascontiguousarray(
            Wq[hs].transpose(1, 0, 2).reshape(E, PER * D)).astype(ndt)
        in_maps.append({
            "xt": xt_b[b],
            "wq": wq_c,
            "wk": np.ascontiguousarray(Wk[g]).astype(ndt),
            "wv": np.ascontiguousarray(Wv[g]).astype(ndt),
            "bq": np.ascontiguousarray(bq[hs].T),
            "bk": np.ascontiguousarray(bk[g][:, None]),
            "bv": np.ascontiguousarray(bv[g][:, None]),
            "mask": mask,
        })
    return in_maps


def assemble(results, s=S):
    """Host-side gather: normalize, transpose, concat heads."""
    out = np.empty((B, s, H * D), dtype=np.float32)
    for c in range(NCORES):
        b, g = c // G, c % G
        ot = results[c]["ot"]                       # [PER, D, s]
        sums = results[c]["sums"].reshape(PER, s)   # [PER, s]
        o = (ot / sums[:, None, :]).transpose(2, 0, 1)   # [s, PER, D]
        out[b, :, g * PER * D:(g + 1) * PER * D] = o.reshape(s, PER * D)
    return out


def _install_trace_shims():
    """Make trace=True work in this container: provide the missing
    antenv.axon_hooks module (ctypes NTFF hook) and disable the
    artifact upload (no bucket access here)."""
    import types
    try:
        import antenv.axon_hooks  # noqa: F401
        have = True
    except ImportError:
        have = False
    if not have:
        from trn_agent_boot.trn_boot import _ntff_profile_via_ctypes
        mod = types.ModuleType("antenv.axon_hooks")
        _hook = _ntff_profile_via_ctypes("/opt/axon/libaxon_pjrt.so")
        mod.get_axon_ntff_profile_hook = lambda: _hook
        mod.set_axon_ntff_profile_hook = lambda h: None
        import antenv
        sys.modules["antenv.axon_hooks"] = mod
        antenv.axon_hooks = mod
    from concourse import bass_utils as bu
    bu.upload_artifacts = lambda tmpdir: f"file://{tmpdir}"


def run(inputs, trace=False, s=S, cdt_name=CDT_NAME):
    from concourse.bass_utils import run_bass_kernel_spmd
    if trace:
        _install_trace_shims()
    key = (s, cdt_name)
    if key not in _CACHE:
        _CACHE[key] = build_nc(s, cdt_name)
    nc = _CACHE[key]
    in_maps = make_in_maps(**inputs, s=s, cdt_name=cdt_name)
    res = run_bass_kernel_spmd(nc, in_maps, list(range(NCORES)), trace=trace)
    return assemble(res.results, s), res


def kernel(**inputs):
    out, _ = run(inputs)
    return out


# revision 24
# speedup vs baseline: 1.0721x; 1.0325x over previous
"""Trainium2 Bass kernel for GQA causal multi-head attention.

Problem: x[B=2,S=2048,E=2048], Wq[H=16,E,D=128], Wk/Wv[G=4,E,D] (+biases).
  q = x@Wq+bq ; k,v = x@Wk/Wv+bk/bv (per group, each group serves 4 heads)
  out = causal_softmax(q k^T / sqrt(D)) @ v, heads concatenated.

Sharding: core c in 0..7 -> (batch b = c//4, group g = c%4).
Each core computes Q for heads 4g..4g+3 of batch b, K/V for group g of
batch b, and full causal attention for those 4 heads. Zero duplication,
no collectives; host does the final transpose/normalize/concat.

Per-core kernel (all layouts chosen so no on-chip transposes of big data):
  xT[E,S] streamed; QT[d,s] = Wq^T xT (PSUM accum over E-tiles), same for
  KT, VT.  V obtained from VT via 16 PE transposes (128x128).
  Attention (transposed-scores formulation): for each q-block of 512 and
  k-sub of 128:  ST[k,q] = (KT chunk)^T-contracted matmul, exp on ACT ->
  PT[k,q] directly (no P transpose!), OT[d,q] += V^T-form matmul(PT),
  denominators via ones-vector M=1 matmul. Causal: skip k>q chunks,
  shrink N on diagonal, single 128x128 triangle mask tile.
Outputs: raw OT[h,d,S] + row sums; host computes (OT/sums)^T.
"""

import math
import os
import sys

sys.path.insert(0, "/opt/trn_rl_repo")

import numpy as np
import ml_dtypes

B, S, E, H, G, D = 2, 2048, 2048, 16, 4, 128
PER = H // G          # 4 q-heads per kv group = heads per core
NCORES = 8
QB = 512              # q block (moving-dim) size
SCALE = 1.0 / math.sqrt(D)

# compute dtype for matmul operands: "bfloat16" | "float32r" | "float32"
CDT_NAME = os.environ.get("BASS_CDT", "bfloat16")

_CACHE = {}


def build_nc(s=S, cdt_name=CDT_NAME):
    """Build the per-core Bass program (same program for all 8 cores).

    v5: projection and attention are interleaved per 512-wide s-chunk so
    the scalar engine's exp work overlaps the PE's projection matmuls.
    Projections run in 3 two-bank PSUM passes ({Q0,Q1},{Q2,Q3},{K,V});
    V is transposed via DMA x-bar (bf16) or PE (f32r); softmax
    denominators accumulate on DVE/GpSimd into per-unit f32r tiles and
    are contracted with a ones vector at the very end.
    """
    import concourse.bass as bass
    import concourse.tile as tile
    from concourse import bacc, mybir
    from concourse.masks import make_identity
    from contextlib import ExitStack

    cdt = getattr(mybir.dt, cdt_name)
    f32 = mybir.dt.float32
    f32r = mybir.dt.float32r
    ET = E // 128          # e-tiles (contraction)
    NSC = s // QB          # s-chunks == q-supers
    EXP = mybir.ActivationFunctionType.Exp

    nc = bacc.Bacc("TRN2", target_bir_lowering=False, debug=False,
                   num_devices=NCORES)

    xt_d = nc.dram_tensor("xt", [E, s], cdt, kind="ExternalInput")
    wq_d = nc.dram_tensor("wq", [E, PER * D], cdt, kind="ExternalInput")
    wk_d = nc.dram_tensor("wk", [E, D], cdt, kind="ExternalInput")
    wv_d = nc.dram_tensor("wv", [E, D], cdt, kind="ExternalInput")
    bq_d = nc.dram_tensor("bq", [D, PER], f32, kind="ExternalInput")
    bk_d = nc.dram_tensor("bk", [D, 1], f32, kind="ExternalInput")
    bv_d = nc.dram_tensor("bv", [D, 1], f32, kind="ExternalInput")
    mask_d = nc.dram_tensor("mask", [128, 128], f32, kind="ExternalInput")
    ot_d = nc.dram_tensor("ot", [PER, D, s], f32, kind="ExternalOutput")
    sums_d = nc.dram_tensor("sums", [1, PER * s], f32, kind="ExternalOutput")

    with tile.TileContext(nc) as tc, ExitStack() as ctx:
        consts = ctx.enter_context(tc.tile_pool(name="consts", bufs=1))
        qkv = ctx.enter_context(tc.tile_pool(name="qkv", bufs=1))
        xpool = ctx.enter_context(tc.tile_pool(name="xtiles", bufs=6))
        ptpool = ctx.enter_context(tc.tile_pool(name="pt", bufs=8))
        opool = ctx.enter_context(tc.tile_pool(name="obuf", bufs=3))
        accp = ctx.enter_context(tc.tile_pool(name="accp", bufs=1))

        # --- constants / weights (wq first + split per e-tile: the first
        # projection pass needs only wq; mask early for the first diagonal)
        wq_sb = consts.tile([128, ET, PER * D], cdt)
        wk_sb = consts.tile([128, ET, D], cdt)
        wv_sb = consts.tile([128, ET, D], cdt)
        wq_r = wq_d[:, :].rearrange("(eo p) m -> p eo m", p=128)
        wk_r = wk_d[:, :].rearrange("(eo p) m -> p eo m", p=128)
        wv_r = wv_d[:, :].rearrange("(eo p) m -> p eo m", p=128)
        bq_sb = consts.tile([128, PER], f32)
        nc.sync.dma_start(bq_sb[:], bq_d[:, :])
        bk_sb = consts.tile([128, 1], f32)
        nc.sync.dma_start(bk_sb[:], bk_d[:, :])
        bv_sb = consts.tile([128, 1], f32)
        nc.sync.dma_start(bv_sb[:], bv_d[:, :])
        mask_sb = consts.tile([128, 128], f32)
        nc.sync.dma_start(mask_sb[:], mask_d[:, :])
        for c4 in range(4):   # wq in 4 chunks, first-needed first
            nc.sync.dma_start(wq_sb[:, c4 * 4:(c4 + 1) * 4, :],
                              wq_r[:, c4 * 4:(c4 + 1) * 4, :])
        nc.sync.dma_start(wk_sb[:], wk_r[:])
        nc.sync.dma_start(wv_sb[:], wv_r[:])
        ones32_sb = consts.tile([128, 1], f32r)
        nc.vector.memset(ones32_sb[:].bitcast(f32), 1.0)
        ident = consts.tile([128, 128], cdt)
        make_identity(nc, ident[:])

        qt_sb = qkv.tile([128, PER, s], cdt)      # QT per head [d, s]
        kt_sb = qkv.tile([128, s], cdt)           # KT [d, s]
        vt_sb = qkv.tile([128, s], cdt)           # VT [d, s]
        v_sb = qkv.tile([128, s], cdt)            # V k-sub j at [:, j*128:...]
        sums_sb = qkv.tile([1, PER * s], f32)
        accs = [(accp.tile([128, QB], f32r, name=f"accd{u}"),
                 accp.tile([128, QB], f32r, name=f"accg{u}"))
                for u in range(PER * NSC)]

        # --- PE warm-up: dummy matmuls on the locally generated identity
        # keep PE busy while the first DMAs land and un-throttle the HAM.
        with tc.tile_pool(name="wpsum", bufs=1, space="PSUM") as wpsum:
            warm = wpsum.tile([128, QB], f32, name="warm")
            nwarm = 72
            for w in range(nwarm):
                nc.tensor.matmul(warm[:, :128], ident[:], ident[:],
                                 start=(w == 0), stop=(w == nwarm - 1))

        mainpools = ctx.enter_context(ExitStack())
        ppsum = mainpools.enter_context(tc.tile_pool(name="ppsum", bufs=2,
                                                     space="PSUM"))
        spsum = mainpools.enter_context(tc.tile_pool(name="spsum", bufs=2,
                                                     space="PSUM"))
        opsum = mainpools.enter_context(tc.tile_pool(name="opsum", bufs=2,
                                                     space="PSUM"))

        def attn_unit(h, qi):
            nj = (qi + 1) * (QB // 128)
            ndiag = QB // 128
            ot_ps = opsum.tile([128, QB], f32, tag="ot", name="ot_ps")
            acc_d, acc_g = accs[h * NSC + qi]
            qsl = qt_sb[:, h, qi * QB:(qi + 1) * QB]

            def pv(j, pt_ap, qoff):
                nc.tensor.matmul(ot_ps[:, qoff:QB],
                                 v_sb[:, j * 128:(j + 1) * 128], pt_ap,
                                 start=(j == 0), stop=(j == nj - 1))

            def accum(j, pt_ap, qoff):
                # two independent chains: even j on DVE, odd j on GpSimd
                eng = nc.vector if j % 2 == 0 else nc.gpsimd
                acc = (acc_d if j % 2 == 0 else acc_g)[:]
                if j < 2:
                    if qoff:
                        eng.memset(acc[:, 0:qoff].bitcast(f32), 0.0)
                    eng.tensor_copy(acc[:, qoff:QB], pt_ap)
                else:
                    eng.tensor_add(acc[:, qoff:QB], acc[:, qoff:QB], pt_ap)

            for jp in range(qi * (QB // 256)):  # off-diagonal pairs
                j0 = jp * 2
                st2 = spsum.tile([128, 2, QB], f32, tag="st", name="st2")
                for m in range(2):
                    nc.tensor.matmul(
                        st2[:, m, :],
                        kt_sb[:, (j0 + m) * 128:(j0 + m + 1) * 128],
                        qsl, start=True, stop=True)
                pt = ptpool.tile([128, 2, QB], cdt, tag="pt", name="pt2")
                nc.scalar.activation(pt[:, :, :], st2[:, :, :], EXP,
                                     scale=SCALE)
                for m in range(2):
                    pv(j0 + m, pt[:, m, :], 0)
                    accum(j0 + m, pt[:, m, :], 0)
            for jd in range(ndiag):             # diagonal singles
                j = qi * ndiag + jd
                kk = j * 128
                qoff = jd * 128
                nq = QB - qoff
                st2 = spsum.tile([128, 2, QB], f32, tag="st", name="st1")
                nc.tensor.matmul(st2[:, 0, :nq], kt_sb[:, kk:kk + 128],
                                 qt_sb[:, h, kk:(qi + 1) * QB],
                                 start=True, stop=True)
                nc.vector.tensor_add(st2[:, 0, 0:128], st2[:, 0, 0:128],
                                     mask_sb[:])
                pt = ptpool.tile([128, 2, QB], cdt, tag="pt", name="pt1")
                nc.scalar.activation(pt[:, 0, :nq], st2[:, 0, :nq], EXP,
                                     scale=SCALE)
                pv(j, pt[:, 0, :nq], qoff)
                accum(j, pt[:, 0, :nq], qoff)
            ot_sbuf = opool.tile([128, QB], f32, tag="osb", name="ot_sbuf")
            nc.vector.tensor_copy(ot_sbuf[:], ot_ps[:])
            nc.sync.dma_start(ot_d[h, :, qi * QB:(qi + 1) * QB], ot_sbuf[:])

        # --- main loop: per s-chunk, projection passes then attention
        PASSES = [("q", 0, 1), ("q", 2, 3), ("kv", None, None)]
        for sc in range(NSC):
            cols = slice(sc * QB, (sc + 1) * QB)
            xt_g = []
            for g4 in range(4):   # 4 e-group chunks of 512KB each
                xg = xpool.tile([128, 4, QB], cdt, tag="xt", name=f"xg{g4}")
                nc.scalar.dma_start(
                    xg[:],
                    xt_d[g4 * 4 * 128:(g4 + 1) * 4 * 128, cols].rearrange(
                        "(eo p) q -> p eo q", p=128))
                xt_g.append(xg)

            def xe(e):
                return xt_g[e // 4][:, e % 4, :]

            for kind, ha, hb in PASSES:
                pa = ppsum.tile([128, QB], f32, tag="pp", name="pa")
                pb = ppsum.tile([128, QB], f32, tag="pp", name="pb")
                for e in range(ET):
                    st_, sp_ = (e == 0), (e == ET - 1)
                    if kind == "q":
                        nc.tensor.matmul(pa[:], wq_sb[:, e, ha * D:(ha + 1) * D],
                                         xe(e), start=st_, stop=sp_)
                        nc.tensor.matmul(pb[:], wq_sb[:, e, hb * D:(hb + 1) * D],
                                         xe(e), start=st_, stop=sp_)
                    else:
                        nc.tensor.matmul(pa[:], wk_sb[:, e, :], xe(e),
                                         start=st_, stop=sp_)
                        nc.tensor.matmul(pb[:], wv_sb[:, e, :], xe(e),
                                         start=st_, stop=sp_)
                if kind == "q":
                    nc.vector.tensor_scalar_add(qt_sb[:, ha, cols], pa[:],
                                                bq_sb[:, ha:ha + 1])
                    nc.vector.tensor_scalar_add(qt_sb[:, hb, cols], pb[:],
                                                bq_sb[:, hb:hb + 1])
                else:
                    nc.vector.tensor_scalar_add(kt_sb[:, cols], pa[:],
                                                bk_sb[:, 0:1])
                    nc.vector.tensor_scalar_add(vt_sb[:, cols], pb[:],
                                                bv_sb[:, 0:1])

            # V tiles for this chunk: 4x 128x128 transposes
            if cdt_name == "bfloat16":
                for jd in range(QB // 128):
                    j = sc * (QB // 128) + jd
                    nc.sync.dma_start(v_sb[:, j * 128:(j + 1) * 128],
                                      vt_sb[:, j * 128:(j + 1) * 128],
                                      transpose=True)
            else:
                for jd in range(QB // 128):
                    j = sc * (QB // 128) + jd
                    vps = ppsum.tile([128, 128], cdt, tag="pp", name="vps")
                    nc.tensor.transpose(vps[:],
                                        vt_sb[:, j * 128:(j + 1) * 128],
                                        ident[:])
                    nc.vector.tensor_copy(v_sb[:, j * 128:(j + 1) * 128],
                                          vps[:])

            for h in range(PER):
                attn_unit(h, qi=sc)

        # --- final: contract exp-sum accumulators with ones (f32r matmuls)
        mainpools.close()
        with tc.tile_pool(name="dpsum", bufs=4, space="PSUM") as dpsum:
            for u in range(PER * NSC):
                sm_ps = dpsum.tile([1, QB], f32, tag="sm", name="sm_ps")
                nc.tensor.matmul(sm_ps[:, :], ones32_sb[:], accs[u][0][:],
                                 start=True, stop=False)
                nc.tensor.matmul(sm_ps[:, :], ones32_sb[:], accs[u][1][:],
                                 start=False, stop=True)
                nc.vector.tensor_copy(sums_sb[0:1, u * QB:(u + 1) * QB],
                                      sm_ps[0:1, :])
        nc.sync.dma_start(sums_d[:, :], sums_sb[:])

    nc.compile()
    return nc


def _np_cdt(cdt_name):
    return {"bfloat16": ml_dtypes.bfloat16,
            "float32r": np.float32,
            "float32": np.float32}[cdt_name]


def make_in_maps(x, Wq, bq, Wk, bk, Wv, bv, s=S, cdt_name=CDT_NAME):
    """Host-side shard + relayout: per-core input dicts."""
    ndt = _np_cdt(cdt_name)
    x = np.asarray(x, dtype=np.float32)
    Wq = np.asarray(Wq, dtype=np.float32)
    bq = np.asarray(bq, dtype=np.float32)
    Wk = np.asarray(Wk, dtype=np.float32)
    bk = np.asarray(bk, dtype=np.float32)
    Wv = np.asarray(Wv, dtype=np.float32)
    bv = np.asarray(bv, dtype=np.float32)
    mask = np.where(np.arange(128)[:, None] > np.arange(128)[None, :],
                    -1e30, 0.0).astype(np.float32)
    in_maps = []
    xt_b = [np.ascontiguousarray(x[b, :s].T).astype(ndt) for b in range(B)]
    for c in range(NCORES):
        b, g = c // G, c % G
        hs = slice(g * PER, (g + 1) * PER)
        wq_c = np.ascontiguousarray(
            Wq[hs].transpose(1, 0, 2).reshape(E, PER * D)).astype(ndt)
        in_maps.append({
            "xt": xt_b[b],
            "wq": wq_c,
            "wk": np.ascontiguousarray(Wk[g]).astype(ndt),
            "wv": np.ascontiguousarray(Wv[g]).astype(ndt),
            "bq": np.ascontiguousarray(bq[hs].T),
            "bk": np.ascontiguousarray(bk[g][:, None]),
            "bv": np.ascontiguousarray(bv[g][:, None]),
            "mask": mask,
        })
    return in_maps


def assemble(results, s=S):
    """Host-side gather: normalize, transpose, concat heads."""
    out = np.empty((B, s, H * D), dtype=np.float32)
    for c in range(NCORES):
        b, g = c // G, c % G
        ot = results[c]["ot"]                       # [PER, D, s]
        sums = results[c]["sums"].reshape(PER, s)   # [PER, s]
        o = (ot / sums[:, None, :]).transpose(2, 0, 1)   # [s, PER, D]
        out[b, :, g * PER * D:(g + 1) * PER * D] = o.reshape(s, PER * D)
    return out


def _install_trace_shims():
    """Make trace=True work in this container: provide the missing
    antenv.axon_hooks module (ctypes NTFF hook) and disable the
    artifact upload (no bucket access here)."""
    import types
    try:
        import antenv.axon_hooks  # noqa: F401
        have = True
    except ImportError:
        have = False
    if not have:
        from trn_agent_boot.trn_boot import _ntff_profile_via_ctypes
        mod = types.ModuleType("antenv.axon_hooks")
        _hook = _ntff_profile_via_ctypes("/opt/axon/libaxon_pjrt.so")
        mod.get_axon_ntff_profile_hook = lambda: _hook
        mod.set_axon_ntff_profile_hook = lambda h: None
        import antenv
        sys.modules["antenv.axon_hooks"] = mod
        antenv.axon_hooks = mod
    from concourse import bass_utils as bu
    bu.upload_artifacts = lambda tmpdir: f"file://{tmpdir}"


def run(inputs, trace=False, s=S, cdt_name=CDT_NAME):
    from concourse.bass_utils import run_bass_kernel_spmd
    if trace:
        _install_trace_shims()
    key = (s, cdt_name)
    if key not in _CACHE:
        _CACHE[key] = build_nc(s, cdt_name)
    nc = _CACHE[key]
    in_maps = make_in_maps(**inputs, s=s, cdt_name=cdt_name)
    res = run_bass_kernel_spmd(nc, in_maps, list(range(NCORES)), trace=trace)
    return assemble(res.results, s), res


def kernel(**inputs):
    out, _ = run(inputs)
    return out


# revision 25
# speedup vs baseline: 1.0727x; 1.0005x over previous
"""Trainium2 Bass kernel for GQA causal multi-head attention.

Problem: x[B=2,S=2048,E=2048], Wq[H=16,E,D=128], Wk/Wv[G=4,E,D] (+biases).
  q = x@Wq+bq ; k,v = x@Wk/Wv+bk/bv (per group, each group serves 4 heads)
  out = causal_softmax(q k^T / sqrt(D)) @ v, heads concatenated.

Sharding: core c in 0..7 -> (batch b = c//4, group g = c%4).
Each core computes Q for heads 4g..4g+3 of batch b, K/V for group g of
batch b, and full causal attention for those 4 heads. Zero duplication,
no collectives; host does the final transpose/normalize/concat.

Per-core kernel (all layouts chosen so no on-chip transposes of big data):
  xT[E,S] streamed; QT[d,s] = Wq^T xT (PSUM accum over E-tiles), same for
  KT, VT.  V obtained from VT via 16 PE transposes (128x128).
  Attention (transposed-scores formulation): for each q-block of 512 and
  k-sub of 128:  ST[k,q] = (KT chunk)^T-contracted matmul, exp on ACT ->
  PT[k,q] directly (no P transpose!), OT[d,q] += V^T-form matmul(PT),
  denominators via ones-vector M=1 matmul. Causal: skip k>q chunks,
  shrink N on diagonal, single 128x128 triangle mask tile.
Outputs: raw OT[h,d,S] + row sums; host computes (OT/sums)^T.
"""

import math
import os
import sys

sys.path.insert(0, "/opt/trn_rl_repo")

import numpy as np
import ml_dtypes

B, S, E, H, G, D = 2, 2048, 2048, 16, 4, 128
PER = H // G          # 4 q-heads per kv group = heads per core
NCORES = 8
QB = 512              # q block (moving-dim) size
SCALE = 1.0 / math.sqrt(D)

# compute dtype for matmul operands: "bfloat16" | "float32r" | "float32"
CDT_NAME = os.environ.get("BASS_CDT", "bfloat16")

_CACHE = {}


def build_nc(s=S, cdt_name=CDT_NAME):
    """Build the per-core Bass program (same program for all 8 cores).

    v5: projection and attention are interleaved per 512-wide s-chunk so
    the scalar engine's exp work overlaps the PE's projection matmuls.
    Projections run in 3 two-bank PSUM passes ({Q0,Q1},{Q2,Q3},{K,V});
    V is transposed via DMA x-bar (bf16) or PE (f32r); softmax
    denominators accumulate on DVE/GpSimd into per-unit f32r tiles and
    are contracted with a ones vector at the very end.
    """
    import concourse.bass as bass
    import concourse.tile as tile
    from concourse import bacc, mybir
    from concourse.masks import make_identity
    from contextlib import ExitStack

    cdt = getattr(mybir.dt, cdt_name)
    f32 = mybir.dt.float32
    f32r = mybir.dt.float32r
    ET = E // 128          # e-tiles (contraction)
    NSC = s // QB          # s-chunks == q-supers
    EXP = mybir.ActivationFunctionType.Exp

    nc = bacc.Bacc("TRN2", target_bir_lowering=False, debug=False,
                   num_devices=NCORES)

    xt_d = nc.dram_tensor("xt", [E, s], cdt, kind="ExternalInput")
    wq_d = nc.dram_tensor("wq", [E, PER * D], cdt, kind="ExternalInput")
    wk_d = nc.dram_tensor("wk", [E, D], cdt, kind="ExternalInput")
    wv_d = nc.dram_tensor("wv", [E, D], cdt, kind="ExternalInput")
    bq_d = nc.dram_tensor("bq", [D, PER], f32, kind="ExternalInput")
    bk_d = nc.dram_tensor("bk", [D, 1], f32, kind="ExternalInput")
    bv_d = nc.dram_tensor("bv", [D, 1], f32, kind="ExternalInput")
    mask_d = nc.dram_tensor("mask", [128, 128], f32, kind="ExternalInput")
    ot_d = nc.dram_tensor("ot", [PER, D, s], f32, kind="ExternalOutput")
    sums_d = nc.dram_tensor("sums", [1, PER * s], f32, kind="ExternalOutput")

    with tile.TileContext(nc) as tc, ExitStack() as ctx:
        consts = ctx.enter_context(tc.tile_pool(name="consts", bufs=1))
        qkv = ctx.enter_context(tc.tile_pool(name="qkv", bufs=1))
        xpool = ctx.enter_context(tc.tile_pool(name="xtiles", bufs=8))
        ptpool = ctx.enter_context(tc.tile_pool(name="pt", bufs=8))
        opool = ctx.enter_context(tc.tile_pool(name="obuf", bufs=3))
        accp = ctx.enter_context(tc.tile_pool(name="accp", bufs=1))

        # --- constants / weights (wq first + split per e-tile: the first
        # projection pass needs only wq; mask early for the first diagonal)
        wq_cs = [consts.tile([128, 4, PER * D], cdt, name=f"wqc{c}")
                 for c in range(ET // 4)]
        wk_sb = consts.tile([128, ET, D], cdt)
        wv_sb = consts.tile([128, ET, D], cdt)
        wq_r = wq_d[:, :].rearrange("(eo p) m -> p eo m", p=128)
        wk_r = wk_d[:, :].rearrange("(eo p) m -> p eo m", p=128)
        wv_r = wv_d[:, :].rearrange("(eo p) m -> p eo m", p=128)
        bq_sb = consts.tile([128, PER], f32)
        nc.sync.dma_start(bq_sb[:], bq_d[:, :])
        bk_sb = consts.tile([128, 1], f32)
        nc.sync.dma_start(bk_sb[:], bk_d[:, :])
        bv_sb = consts.tile([128, 1], f32)
        nc.sync.dma_start(bv_sb[:], bv_d[:, :])
        mask_sb = consts.tile([128, 128], f32)
        nc.sync.dma_start(mask_sb[:], mask_d[:, :])
        for c4 in range(4):   # wq in 4 chunks, first-needed first
            nc.sync.dma_start(wq_cs[c4][:],
                              wq_r[:, c4 * 4:(c4 + 1) * 4, :])
        nc.sync.dma_start(wk_sb[:], wk_r[:])
        nc.sync.dma_start(wv_sb[:], wv_r[:])
        ones32_sb = consts.tile([128, 1], f32r)
        nc.vector.memset(ones32_sb[:].bitcast(f32), 1.0)
        ident = consts.tile([128, 128], cdt)
        make_identity(nc, ident[:])

        qt_sb = qkv.tile([128, PER, s], cdt)      # QT per head [d, s]
        kt_sb = qkv.tile([128, s], cdt)           # KT [d, s]
        vt_sb = qkv.tile([128, s], cdt)           # VT [d, s]
        v_sb = qkv.tile([128, s], cdt)            # V k-sub j at [:, j*128:...]
        sums_sb = qkv.tile([1, PER * s], f32)
        accs = [(accp.tile([128, QB], f32r, name=f"accd{u}"),
                 accp.tile([128, QB], f32r, name=f"accg{u}"))
                for u in range(PER * NSC)]

        # --- PE warm-up: dummy matmuls on the locally generated identity
        # keep PE busy while the first DMAs land and un-throttle the HAM.
        with tc.tile_pool(name="wpsum", bufs=1, space="PSUM") as wpsum:
            warm = wpsum.tile([128, QB], f32, name="warm")
            nwarm = 72
            for w in range(nwarm):
                nc.tensor.matmul(warm[:, :128], ident[:], ident[:],
                                 start=(w == 0), stop=(w == nwarm - 1))

        mainpools = ctx.enter_context(ExitStack())
        ppsum = mainpools.enter_context(tc.tile_pool(name="ppsum", bufs=2,
                                                     space="PSUM"))
        spsum = mainpools.enter_context(tc.tile_pool(name="spsum", bufs=2,
                                                     space="PSUM"))
        opsum = mainpools.enter_context(tc.tile_pool(name="opsum", bufs=2,
                                                     space="PSUM"))

        def attn_unit(h, qi):
            nj = (qi + 1) * (QB // 128)
            ndiag = QB // 128
            ot_ps = opsum.tile([128, QB], f32, tag="ot", name="ot_ps")
            acc_d, acc_g = accs[h * NSC + qi]
            qsl = qt_sb[:, h, qi * QB:(qi + 1) * QB]

            def pv(j, pt_ap, qoff):
                nc.tensor.matmul(ot_ps[:, qoff:QB],
                                 v_sb[:, j * 128:(j + 1) * 128], pt_ap,
                                 start=(j == 0), stop=(j == nj - 1))

            inited = {"d": False, "g": False}

            def accum(j, pt_ap, qoff):
                # independent chains, 2/3 on DVE, 1/3 on the slower GpSimd
                gps = (j % 3 == 2)
                eng = nc.gpsimd if gps else nc.vector
                acc = (acc_g if gps else acc_d)[:]
                key = "g" if gps else "d"
                if not inited[key]:
                    inited[key] = True
                    if qoff:
                        eng.memset(acc[:, 0:qoff].bitcast(f32), 0.0)
                    eng.tensor_copy(acc[:, qoff:QB], pt_ap)
                else:
                    eng.tensor_add(acc[:, qoff:QB], acc[:, qoff:QB], pt_ap)

            for jp in range(qi * (QB // 256)):  # off-diagonal pairs
                j0 = jp * 2
                st2 = spsum.tile([128, 2, QB], f32, tag="st", name="st2")
                for m in range(2):
                    nc.tensor.matmul(
                        st2[:, m, :],
                        kt_sb[:, (j0 + m) * 128:(j0 + m + 1) * 128],
                        qsl, start=True, stop=True)
                pt = ptpool.tile([128, 2, QB], cdt, tag="pt", name="pt2")
                nc.scalar.activation(pt[:, :, :], st2[:, :, :], EXP,
                                     scale=SCALE)
                for m in range(2):
                    pv(j0 + m, pt[:, m, :], 0)
                    accum(j0 + m, pt[:, m, :], 0)
            for jd in range(ndiag):             # diagonal singles
                j = qi * ndiag + jd
                kk = j * 128
                qoff = jd * 128
                nq = QB - qoff
                st2 = spsum.tile([128, 2, QB], f32, tag="st", name="st1")
                nc.tensor.matmul(st2[:, 0, :nq], kt_sb[:, kk:kk + 128],
                                 qt_sb[:, h, kk:(qi + 1) * QB],
                                 start=True, stop=True)
                nc.vector.tensor_add(st2[:, 0, 0:128], st2[:, 0, 0:128],
                                     mask_sb[:])
                pt = ptpool.tile([128, 2, QB], cdt, tag="pt", name="pt1")
                nc.scalar.activation(pt[:, 0, :nq], st2[:, 0, :nq], EXP,
                                     scale=SCALE)
                pv(j, pt[:, 0, :nq], qoff)
                accum(j, pt[:, 0, :nq], qoff)
            ot_sbuf = opool.tile([128, QB], f32, tag="osb", name="ot_sbuf")
            nc.vector.tensor_copy(ot_sbuf[:], ot_ps[:])
            nc.sync.dma_start(ot_d[h, :, qi * QB:(qi + 1) * QB], ot_sbuf[:])

        # --- main loop: per s-chunk, projection passes then attention.
        # xt chunks for s-chunk sc+1 are DMA'd at the top of iteration sc.
        PASSES = [("q", 0, 1), ("q", 2, 3), ("kv", None, None)]

        def load_xg(sc):
            cols = slice(sc * QB, (sc + 1) * QB)
            tiles = []
            for g4 in range(4):   # 4 e-group chunks of 512KB each
                xg = xpool.tile([128, 4, QB], cdt, tag="xt", name=f"xg{g4}")
                nc.scalar.dma_start(
                    xg[:],
                    xt_d[g4 * 4 * 128:(g4 + 1) * 4 * 128, cols].rearrange(
                        "(eo p) q -> p eo q", p=128))
                tiles.append(xg)
            return tiles

        xg_next = load_xg(0)
        for sc in range(NSC):
            cols = slice(sc * QB, (sc + 1) * QB)
            xt_g = xg_next
            if sc + 1 < NSC:
                xg_next = load_xg(sc + 1)

            def xe(e):
                return xt_g[e // 4][:, e % 4, :]

            for kind, ha, hb in PASSES:
                pa = ppsum.tile([128, QB], f32, tag="pp", name="pa")
                pb = ppsum.tile([128, QB], f32, tag="pp", name="pb")
                for e in range(ET):
                    st_, sp_ = (e == 0), (e == ET - 1)
                    if kind == "q":
                        wqe = wq_cs[e // 4][:, e % 4, :]
                        nc.tensor.matmul(pa[:], wqe[:, ha * D:(ha + 1) * D],
                                         xe(e), start=st_, stop=sp_)
                        nc.tensor.matmul(pb[:], wqe[:, hb * D:(hb + 1) * D],
                                         xe(e), start=st_, stop=sp_)
                    else:
                        nc.tensor.matmul(pa[:], wk_sb[:, e, :], xe(e),
                                         start=st_, stop=sp_)
                        nc.tensor.matmul(pb[:], wv_sb[:, e, :], xe(e),
                                         start=st_, stop=sp_)
                if kind == "q":
                    nc.vector.tensor_scalar_add(qt_sb[:, ha, cols], pa[:],
                                                bq_sb[:, ha:ha + 1])
                    nc.vector.tensor_scalar_add(qt_sb[:, hb, cols], pb[:],
                                                bq_sb[:, hb:hb + 1])
                else:
                    nc.vector.tensor_scalar_add(kt_sb[:, cols], pa[:],
                                                bk_sb[:, 0:1])
                    nc.vector.tensor_scalar_add(vt_sb[:, cols], pb[:],
                                                bv_sb[:, 0:1])

            # V tiles for this chunk: 4x 128x128 transposes
            if cdt_name == "bfloat16":
                for jd in range(QB // 128):
                    j = sc * (QB // 128) + jd
                    nc.sync.dma_start(v_sb[:, j * 128:(j + 1) * 128],
                                      vt_sb[:, j * 128:(j + 1) * 128],
                                      transpose=True)
            else:
                for jd in range(QB // 128):
                    j = sc * (QB // 128) + jd
                    vps = ppsum.tile([128, 128], cdt, tag="pp", name="vps")
                    nc.tensor.transpose(vps[:],
                                        vt_sb[:, j * 128:(j + 1) * 128],
                                        ident[:])
                    nc.vector.tensor_copy(v_sb[:, j * 128:(j + 1) * 128],
                                          vps[:])

            for h in range(PER):
                attn_unit(h, qi=sc)

        # --- final: contract exp-sum accumulators with ones (f32r matmuls)
        mainpools.close()
        with tc.tile_pool(name="dpsum", bufs=4, space="PSUM") as dpsum:
            for u in range(PER * NSC):
                sm_ps = dpsum.tile([1, QB], f32, tag="sm", name="sm_ps")
                nc.tensor.matmul(sm_ps[:, :], ones32_sb[:], accs[u][0][:],
                                 start=True, stop=False)
                nc.tensor.matmul(sm_ps[:, :], ones32_sb[:], accs[u][1][:],
                                 start=False, stop=True)
                nc.vector.tensor_copy(sums_sb[0:1, u * QB:(u + 1) * QB],
                                      sm_ps[0:1, :])
        nc.sync.dma_start(sums_d[:, :], sums_sb[:])

    nc.compile()
    return nc


def _np_cdt(cdt_name):
    return {"bfloat16": ml_dtypes.bfloat16,
            "float32r": np.float32,
            "float32": np.float32}[cdt_name]


def make_in_maps(x, Wq, bq, Wk, bk, Wv, bv, s=S, cdt_name=CDT_NAME):
    """Host-side shard + relayout: per-core input dicts."""
    ndt = _np_cdt(cdt_name)
    x = np.asarray(x, dtype=np.float32)
    Wq = np.asarray(Wq, dtype=np.float32)
    bq = np.asarray(bq, dtype=np.float32)
    Wk = np.asarray(Wk, dtype=np.float32)
    bk = np.asarray(bk, dtype=np.float32)
    Wv = np.asarray(Wv, dtype=np.float32)
    bv = np.asarray(bv, dtype=np.float32)
    mask = np.where(np.arange(128)[:, None] > np.arange(128)[None, :],
                    -1e30, 0.0).astype(np.float32)
    in_maps = []
    xt_b = [np.ascontiguousarray(x[b, :s].T).astype(ndt) for b in range(B)]
    for c in range(NCORES):
        b, g = c // G, c % G
        hs = slice(g * PER, (g + 1) * PER)
        wq_c = np.ascontiguousarray(
            Wq[hs].transpose(1, 0, 2).reshape(E, PER * D)).astype(ndt)
        in_maps.append({
            "xt": xt_b[b],
            "wq": wq_c,
            "wk": np.ascontiguousarray(Wk[g]).astype(ndt),
            "wv": np.ascontiguousarray(Wv[g]).astype(ndt),
            "bq": np.ascontiguousarray(bq[hs].T),
            "bk": np.ascontiguousarray(bk[g][:, None]),
            "bv": np.ascontiguousarray(bv[g][:, None]),
            "mask": mask,
        })
    return in_maps


def assemble(results, s=S):
    """Host-side gather: normalize, transpose, concat heads."""
    out = np.empty((B, s, H * D), dtype=np.float32)
    for c in range(NCORES):
        b, g = c // G, c % G
        ot = results[c]["ot"]                       # [PER, D, s]
        sums = results[c]["sums"].reshape(PER, s)   # [PER, s]
        o = (ot / sums[:, None, :]).transpose(2, 0, 1)   # [s, PER, D]
        out[b, :, g * PER * D:(g + 1) * PER * D] = o.reshape(s, PER * D)
    return out


def _install_trace_shims():
    """Make trace=True work in this container: provide the missing
    antenv.axon_hooks module (ctypes NTFF hook) and disable the
    artifact upload (no bucket access here)."""
    import types
    try:
        import antenv.axon_hooks  # noqa: F401
        have = True
    except ImportError:
        have = False
    if not have:
        from trn_agent_boot.trn_boot import _ntff_profile_via_ctypes
        mod = types.ModuleType("antenv.axon_hooks")
        _hook = _ntff_profile_via_ctypes("/opt/axon/libaxon_pjrt.so")
        mod.get_axon_ntff_profile_hook = lambda: _hook
        mod.set_axon_ntff_profile_hook = lambda h: None
        import antenv
        sys.modules["antenv.axon_hooks"] = mod
        antenv.axon_hooks = mod
    from concourse import bass_utils as bu
    bu.upload_artifacts = lambda tmpdir: f"file://{tmpdir}"


def run(inputs, trace=False, s=S, cdt_name=CDT_NAME):
    from concourse.bass_utils import run_bass_kernel_spmd
    if trace:
        _install_trace_shims()
    key = (s, cdt_name)
    if key not in _CACHE:
        _CACHE[key] = build_nc(s, cdt_name)
    nc = _CACHE[key]
    in_maps = make_in_maps(**inputs, s=s, cdt_name=cdt_name)
    res = run_bass_kernel_spmd(nc, in_maps, list(range(NCORES)), trace=trace)
    return assemble(res.results, s), res


def kernel(**inputs):
    out, _ = run(inputs)
    return out
